# revision 12
# baseline (speedup 1.0000x reference)
"""GAT layer on 8 Trainium2 NeuronCores (Bass/Tile), v2.

Strategy (target-per-partition, fp8 table rows, 4-queue gathers):
  - Targets sharded across 8 cores (12500 each); per core targets are
    Morton-clustered into 98 blocks of 128 (block b target v on SBUF
    partition v), then sorted by total edge count inside each block so
    per-call trailing -1 indices trim real descriptors.
  - Phase B builds a DRAM table row per node: [p fp8e4m3(128B) |
    alpha_src f32x8 (32B) | pad] = 256B rows via PE matmuls
    (stationary = xT k-block, moving = [W_proj | W_proj@Ablk]); PSUM is
    drained by DVE in 3-matmul batches (fp8 cast + f32 alpha copy).
  - Phase C processes blocks in groups (slot budget SW_BUD): per-edge
    rows pulled by ANT dma_gather on all 4 SWDGE queues (greedy
    W-balanced queue assignment; each queue owns a Q7 core pair, so
    descriptor generation runs up to 4-wide). Exact per-call gcnt =
    last real slot + 1; mid-stream pads use the sentinel row
    (alpha=-80 => E~1e-7), trailing pads are -1 (skipped by the DGE).
  - Compute batched per group: s = alpha + beta[t] (per block),
    exp(lrelu) via one ACT Exp after a DVE leaky-max; M = p*E in one
    fp8 x bf16 group multiply; per-block pairwise-tree slot sum; ELU
    via DVE min/max + one ACT Exp; one output DMA per group.
"""

import os
import sys

sys.path.insert(0, "/opt/trn_rl_repo")

import numpy as np
from contextlib import ExitStack

import concourse.bass as bass
import concourse.bacc as bacc
import concourse.tile as tile
from concourse import mybir
from concourse._compat import cdiv
from concourse.bass_utils import run_bass_kernel_spmd
from concourse.library_config import mlp

N_NODES = 100000
N_EDGES = 1600000
IN_F = 128
H = 8
F = 16
HF = H * F  # 128
NEG_SLOPE = 0.2
N_CORES = 8
TGT_PER_CORE = N_NODES // N_CORES  # 12500
N_BLOCKS = cdiv(TGT_PER_CORE, 128)  # 98
TGT_PAD = N_BLOCKS * 128  # 12544

STRIP = 2048
N_STRIPS = 49
NPAD = STRIP * N_STRIPS  # 100352
N_CHUNKS = 4
CH_ROWBASE = [0, 32768, 65536, 98304]
CH_ROWS = [32768, 32768, 32768, 2052]  # ch3: 2048 + 3 dups + sentinel
ROW = 256  # bytes per table row (fp8 p 128 | f32 alpha 32 | pad 96)
ALPHA_SENT = -80.0
STOLEN = [32767, 65535, 98303]
DUP_LOCAL3 = [2048, 2049, 2050]
SENT_LOCAL = [32767, 32767, 32767, 2051]
GCNT_PAD = 400  # 98*4 calls padded
SW_BUD = 76  # max gather slots per group
NB_CAP = 10  # max blocks per group

_COMPILED = {}


def _row_of(n):
    s = n // STRIP
    w = n % STRIP
    return s * STRIP + (w % 128) * 16 + (w // 128)


def _to_bf16(a):
    import ml_dtypes
    return np.ascontiguousarray(np.asarray(a, np.float32)).astype(
        ml_dtypes.bfloat16)


def _groups(W):
    """Greedy pack consecutive blocks into groups with sum(Wsum) <= SW_BUD."""
    Wsum = W.sum(axis=1)
    assert Wsum.max() <= SW_BUD, f"block Wsum {Wsum.max()} exceeds {SW_BUD}"
    groups = []
    cur, acc = [], 0
    for b in range(N_BLOCKS):
        if cur and (acc + Wsum[b] > SW_BUD or len(cur) >= NB_CAP):
            groups.append(cur)
            cur, acc = [], 0
        cur.append(b)
        acc += int(Wsum[b])
    if cur:
        groups.append(cur)
    return groups


def _call_order(W):
    """Emission order of gather calls: per group, chunk-major."""
    order = []
    for grp in _groups(W):
        for ch in range(N_CHUNKS):
            for b in grp:
                order.append((b, ch))
    return order


def _host_prep(x, edge_index, W_proj, W_skip, a_src, a_tgt):
    """Pure index/layout prep. Returns (common, per_core list)."""
    x = np.asarray(x, np.float32)
    ei = np.asarray(edge_index)
    src = ei[0].astype(np.int64)
    tgt = ei[1].astype(np.int64)

    xT = np.zeros((IN_F, NPAD), np.float32)
    xT[:, :N_NODES] = x.T
    xT16 = _to_bf16(xT)

    Wp = np.asarray(W_proj, np.float64)
    Ws = np.asarray(W_skip, np.float64)
    asr = np.asarray(a_src, np.float64).reshape(H, F)
    atg = np.asarray(a_tgt, np.float64).reshape(H, F)
    Ablk = np.zeros((HF, H))
    Bblk = np.zeros((HF, H))
    for h in range(H):
        Ablk[h * F:(h + 1) * F, h] = asr[h]
        Bblk[h * F:(h + 1) * F, h] = atg[h]
    pack0 = _to_bf16(np.concatenate([Wp, Wp @ Ablk], axis=1).astype(np.float32))
    pack2 = _to_bf16(np.concatenate([Ws, Wp @ Bblk], axis=1).astype(np.float32))

    chunk_of = np.minimum(src // 32768, 3)
    rows = _row_of(src)
    idxval = rows - np.take(np.array(CH_ROWBASE, np.int64), chunk_of)
    for i, xn in enumerate(STOLEN):
        m = src == xn
        chunk_of[m] = 3
        idxval[m] = DUP_LOCAL3[i]
    sent_idx = np.array(SENT_LOCAL, np.int64)

    cores = []
    for c in range(N_CORES):
        lo, hi = c * TGT_PER_CORE, (c + 1) * TGT_PER_CORE
        m = (tgt >= lo) & (tgt < hi)
        s_idx = idxval[m]
        s_ch = chunk_of[m]
        t_loc = (tgt[m] - lo).astype(np.int64)
        cnt = np.zeros((TGT_PER_CORE, N_CHUNKS), np.int32)
        np.add.at(cnt, (t_loc, s_ch), 1)
        # Morton order on the 4-chunk count vector clusters targets that
        # are similar in all chunks, shrinking per-chunk block maxima.
        cc = np.minimum(cnt, 31).astype(np.int64)
        mkey = np.zeros(TGT_PER_CORE, np.int64)
        for bit in range(5):
            for ch in range(N_CHUNKS):
                mkey |= ((cc[:, ch] >> bit) & 1) << (bit * 4 + ch)
        order = np.argsort(-mkey, kind="stable")
        # Within each block sort by total count desc: real edges cluster
        # at low partition ids, so the trailing -1 trim drops more slots.
        tot = cnt.sum(axis=1)
        for b in range(N_BLOCKS):
            sl = slice(b * 128, min((b + 1) * 128, TGT_PER_CORE))
            seg = order[sl]
            order[sl] = seg[np.argsort(-tot[seg], kind="stable")]
        rank = np.empty(TGT_PER_CORE, np.int64)
        rank[order] = np.arange(TGT_PER_CORE)
        cnt_pad = np.zeros((TGT_PAD, N_CHUNKS), np.int32)
        cnt_pad[:TGT_PER_CORE] = cnt[order]
        wc = np.maximum(cnt_pad.reshape(N_BLOCKS, 128, N_CHUNKS).max(axis=1), 1)
        cores.append(dict(order=order, rank=rank, wc=wc,
                          s_idx=s_idx, s_ch=s_ch, t_loc=t_loc))

    W = np.zeros((N_BLOCKS, N_CHUNKS), np.int32)
    for c in range(N_CORES):
        W = np.maximum(W, cores[c]["wc"])

    call_order = _call_order(W)

    per_core = []
    for c in range(N_CORES):
        d = cores[c]
        rk = d["rank"][d["t_loc"]]
        eo = np.lexsort((d["s_ch"], rk))
        rk_s = rk[eo]
        ch_s = d["s_ch"][eo]
        sl_s = d["s_idx"][eo]
        key = rk_s * N_CHUNKS + ch_s
        uk = np.unique(key)
        firsts = np.searchsorted(key, uk)
        dpos = np.arange(len(key)) - firsts[np.searchsorted(uk, key)]
        idx_cols = [None] * (N_BLOCKS * N_CHUNKS)
        gcnt_bc = np.zeros((N_BLOCKS, N_CHUNKS), np.int32)
        for b in range(N_BLOCKS):
            for ch in range(N_CHUNKS):
                w = int(W[b, ch])
                arr = np.full((w, 128), -1, np.int64)
                mm = (rk_s // 128 == b) & (ch_s == ch)
                if mm.any():
                    p = (rk_s[mm] % 128).astype(np.int64)
                    dd = dpos[mm]
                    arr[dd, p] = sl_s[mm]
                flat = arr.reshape(-1)  # j = d*128 + p
                used = np.nonzero(flat >= 0)[0]
                L = int(used.max()) + 1 if len(used) else 0
                padm = (flat < 0) & (np.arange(w * 128) < L)
                flat[padm] = sent_idx[ch]
                gcnt_bc[b, ch] = L
                wrap = flat.reshape(-1, 16).T  # [16, 8w]
                idx_cols[b * N_CHUNKS + ch] = np.tile(wrap, (8, 1))
        idxs = np.concatenate(idx_cols, axis=1).astype(np.int16)
        gcnt = np.zeros(GCNT_PAD, np.int32)
        for i, (b, ch) in enumerate(call_order):
            gcnt[i] = gcnt_bc[b, ch]

        perm = d["order"]
        xTp = np.zeros((IN_F, TGT_PAD), np.float32)
        xTp[:, :TGT_PER_CORE] = x[perm + c * TGT_PER_CORE].T
        per_core.append(dict(idxs=idxs, xTperm=_to_bf16(xTp), perm=perm,
                             gcnt=gcnt.reshape(1, -1)))

    common = dict(xT=xT16, pack0=pack0, pack2=pack2, W=W)
    return common, per_core


def _build_program(W):
    nc = bacc.Bacc("TRN2", debug=False, num_devices=N_CORES,
                   num_swdge_queues=4)
    f32 = mybir.dt.float32
    bf16 = mybir.dt.bfloat16
    f8 = mybir.dt.float8e4
    i16 = mybir.dt.int16

    C_total = int(8 * W.sum())
    groups = _groups(W)
    Wsum = W.sum(axis=1)
    # block -> (group index, slot offset within group)
    blk_off = {}
    grp_sw = []
    for gi, grp in enumerate(groups):
        off = 0
        for b in grp:
            blk_off[b] = (gi, off)
            off += int(Wsum[b])
        grp_sw.append(off)
    # idx column offsets in block-major layout (matches host idxs build)
    col_off = np.concatenate(([0], np.cumsum(8 * W.reshape(-1))))

    xT_d = nc.dram_tensor("xT", [IN_F, NPAD], bf16, kind="ExternalInput").ap()
    xTperm_d = nc.dram_tensor("xTperm", [IN_F, TGT_PAD], bf16,
                              kind="ExternalInput").ap()
    pack0_d = nc.dram_tensor("pack0", [IN_F, HF + H], bf16,
                             kind="ExternalInput").ap()
    pack2_d = nc.dram_tensor("pack2", [IN_F, HF + H], bf16,
                             kind="ExternalInput").ap()
    idxs_d = nc.dram_tensor("idxs", [128, C_total], i16,
                            kind="ExternalInput").ap()
    gcnt_d = nc.dram_tensor("gcnt", [1, GCNT_PAD], mybir.dt.int32,
                            kind="ExternalInput").ap()
    out_d = nc.dram_tensor("out", [TGT_PAD, HF], f32,
                           kind="ExternalOutput").ap()
    tables = [nc.dram_tensor(f"table{ch}", [CH_ROWS[ch], ROW],
                             f8).ap() for ch in range(N_CHUNKS)]

    with tile.TileContext(nc) as tc, ExitStack() as ctx:
        consts = ctx.enter_context(tc.tile_pool(name="consts", bufs=1))
        stg = ctx.enter_context(tc.tile_pool(name="stg", bufs=2))
        rowp = ctx.enter_context(tc.tile_pool(name="rowp", bufs=2))
        gpool = ctx.enter_context(tc.tile_pool(name="gpool", bufs=3))
        mpool = ctx.enter_context(tc.tile_pool(name="mpool", bufs=2))
        sepool = ctx.enter_context(tc.tile_pool(name="sepool", bufs=2))
        opool = ctx.enter_context(tc.tile_pool(name="opool", bufs=2))
        bskp = ctx.enter_context(tc.tile_pool(name="bskp", bufs=2))
        dpool = ctx.enter_context(tc.tile_pool(name="dpool", bufs=2))
        psA = ctx.enter_context(tc.tile_pool(name="psA", bufs=3, space="PSUM"))
        psC = ctx.enter_context(tc.tile_pool(name="psC", bufs=3, space="PSUM"))
        idxp = ctx.enter_context(tc.tile_pool(name="idxp", bufs=2))

        nc.gpsimd.load_library(mlp)

        # --- constants ------------------------------------------------
        pack0_t = consts.tile([IN_F, HF + H], bf16)
        nc.sync.dma_start(out=pack0_t[:], in_=pack0_d[:])
        pack2_t = consts.tile([IN_F, HF + H], bf16)
        nc.sync.dma_start(out=pack2_t[:], in_=pack2_d[:])
        xTperm_t = consts.tile([IN_F, TGT_PAD], bf16)
        nc.sync.dma_start(out=xTperm_t[:], in_=xTperm_d[:])
        gcnt_t = consts.tile([1, GCNT_PAD], mybir.dt.int32)
        nc.sync.dma_start(out=gcnt_t[:], in_=gcnt_d[:])
        cregs = [nc.gpsimd.alloc_register(f"gc{i}") for i in range(16)]
        sent_t = consts.tile([1, ROW], f8)
        nc.vector.memset(sent_t[:, 0:HF], 0.0)
        nc.vector.memset(sent_t[:, HF:HF + 4 * H].bitcast(f32), ALPHA_SENT)
        nc.vector.memset(sent_t[:, HF + 4 * H:].bitcast(f32), 0.0)

        # --- Phase B: build per-node table rows -----------------------
        for s in range(N_STRIPS):
            ch = min(s // 16, 3)
            sl = s - 16 * ch
            xs = stg.tile([IN_F, STRIP], bf16, tag="xs")
            nc.scalar.dma_start(out=xs[:],
                                in_=xT_d[:, s * STRIP:(s + 1) * STRIP])
            rb = rowp.tile([128, 16, ROW], f8, tag="rb")
            for k3 in range(0, 16, 3):
                kn = min(3, 16 - k3)
                pa = psA.tile([128, 3, HF + H], f32, space="PSUM", tag="pa")
                for t in range(kn):
                    nc.tensor.matmul(
                        out=pa[:, t, :],
                        lhsT=xs[:, (k3 + t) * 128:(k3 + t + 1) * 128],
                        rhs=pack0_t[:], start=True, stop=True)
                # p-cast on the otherwise-idle ACT engine (all B Copies
                # precede all C Exps in the ACT queue: one table switch)
                nc.scalar.activation(out=rb[:, k3:k3 + kn, 0:HF],
                                     in_=pa[:, 0:kn, 0:HF],
                                     func=mybir.ActivationFunctionType.Copy)
                nc.vector.tensor_copy(
                    out=rb[:, k3:k3 + kn, HF:HF + 4 * H].bitcast(f32),
                    in_=pa[:, 0:kn, HF:HF + H])
            nc.sync.dma_start(
                out=tables[ch][sl * STRIP:(sl + 1) * STRIP, :].rearrange(
                    "(p k) e -> p k e", k=16),
                in_=rb[:])
            if ch < 3 and sl == 15:
                nc.sync.dma_start(
                    out=tables[3][DUP_LOCAL3[ch]:DUP_LOCAL3[ch] + 1, :],
                    in_=tables[ch][32767:32768, :])
                nc.sync.dma_start(out=tables[ch][32767:32768, :],
                                  in_=sent_t[:])
            elif ch == 3:
                nc.sync.dma_start(out=tables[3][2051:2052, :], in_=sent_t[:])

        # --- Phase C: grouped edge processing -------------------------
        qload = [0, 0, 0, 0]
        call_i = 0
        for gi, grp in enumerate(groups):
            nb = len(grp)
            swg = grp_sw[gi]
            b0, bN = grp[0], grp[-1]
            g0 = int(col_off[b0 * N_CHUNKS])
            g1 = int(col_off[(bN + 1) * N_CHUNKS])
            idx_t = idxp.tile([128, 8 * SW_BUD], i16, tag="idxg")
            nc.sync.dma_start(out=idx_t[:, 0:g1 - g0], in_=idxs_d[:, g0:g1])

            G = gpool.tile([128, SW_BUD, ROW], f8, tag="G")
            M = mpool.tile([128, SW_BUD, HF], bf16, tag="M")
            if gi < 3:  # once per gpool buffer (pristine SBUF may hold NaNs)
                nc.vector.memset(G[:], 0.0)
            # preset alpha region (covers DGE-skipped slots)
            nc.vector.memset(G[:, 0:swg, HF:HF + 4 * H].bitcast(f32),
                             ALPHA_SENT)

            # skip | beta matmuls, drained to an SBUF group tile
            bskG = bskp.tile([128, NB_CAP, HF + H], f32, tag="bskG")
            for j, b in enumerate(grp):
                sk_ps = psC.tile([128, HF + H], f32, space="PSUM", tag="sk")
                nc.tensor.matmul(out=sk_ps[:],
                                 lhsT=xTperm_t[:, b * 128:(b + 1) * 128],
                                 rhs=pack2_t[:], start=True, stop=True)
                nc.vector.tensor_copy(out=bskG[:, j, :], in_=sk_ps[:])

            # gathers: chunk-major, queue balanced by W
            for ch in range(N_CHUNKS):
                for b in grp:
                    w = int(W[b, ch])
                    doff = blk_off[b][1] + int(W[b, :ch].sum())
                    if call_i % 16 == 0:
                        nc.gpsimd.reg_load(
                            cregs, gcnt_t[0:1, call_i:call_i + 16])
                    c0 = int(col_off[b * N_CHUNKS + ch]) - g0
                    q = min(range(4), key=lambda i: qload[i])
                    qload[q] += w
                    nc.gpsimd.dma_gather(
                        G[:, doff:doff + w, :],
                        tables[ch][:],
                        idx_t[:, c0:c0 + 8 * w],
                        128 * w, cregs[call_i % 16], ROW,
                        single_packet=False,
                        queue_num=q,
                    )
                    call_i += 1

            # scores: s = alpha + beta[t] per block, then group lrelu+exp
            s_t = sepool.tile([128, SW_BUD, H], f32, tag="s")
            z2 = sepool.tile([128, SW_BUD, H], f32, tag="z2")
            E_t = sepool.tile([128, SW_BUD, H], bf16, tag="E")
            for j, b in enumerate(grp):
                off = blk_off[b][1]
                sw = int(Wsum[b])
                nc.vector.tensor_tensor(
                    out=s_t[:, off:off + sw, :],
                    in0=G[:, off:off + sw, HF:HF + 4 * H].bitcast(f32),
                    in1=bskG[:, j, HF:HF + H].unsqueeze(1).to_broadcast(
                        [128, sw, H]),
                    op=mybir.AluOpType.add)
            nc.vector.tensor_scalar(
                out=z2[:, 0:swg, :], in0=s_t[:, 0:swg, :],
                scalar1=NEG_SLOPE, scalar2=None, op0=mybir.AluOpType.mult)
            nc.vector.tensor_tensor(
                out=s_t[:, 0:swg, :], in0=s_t[:, 0:swg, :],
                in1=z2[:, 0:swg, :], op=mybir.AluOpType.max)
            nc.scalar.activation(out=E_t[:, 0:swg, :], in_=s_t[:, 0:swg, :],
                                 func=mybir.ActivationFunctionType.Exp)

            # M = p * E (one group-wide fp8 x bf16 multiply)
            nc.vector.tensor_tensor(
                out=M[:, 0:swg, :].rearrange("p w (h f) -> p w h f", h=H),
                in0=G[:, 0:swg, 0:HF].rearrange("p w (h f) -> p w h f", h=H),
                in1=E_t[:, 0:swg, :].unsqueeze(3).to_broadcast(
                    [128, swg, H, F]),
                op=mybir.AluOpType.mult)

            # per-block slot reduction; divide/skip batched per group
            O = opool.tile([128, NB_CAP, HF], f32, tag="O")
            T1 = opool.tile([128, NB_CAP, HF], f32, tag="T1")
            Dg = dpool.tile([128, NB_CAP, H], f32, tag="Dg")
            for j, b in enumerate(grp):
                off = blk_off[b][1]
                sw = int(Wsum[b])
                Mb = M[:, off:off + sw, :]
                L = sw
                while L > 2:
                    hh = L // 2
                    nc.vector.tensor_tensor(
                        out=Mb[:, 0:hh, :], in0=Mb[:, 0:hh, :],
                        in1=Mb[:, L - hh:L, :], op=mybir.AluOpType.add)
                    L = L - hh
                if L == 2:
                    nc.vector.tensor_tensor(
                        out=O[:, j, :].unsqueeze(1), in0=Mb[:, 0:1, :],
                        in1=Mb[:, 1:2, :], op=mybir.AluOpType.add)
                else:
                    nc.vector.tensor_copy(out=O[:, j, :].unsqueeze(1),
                                          in_=Mb[:, 0:1, :])
                nc.vector.tensor_reduce(
                    out=Dg[:, j, :],
                    in_=E_t[:, off:off + sw, :].transpose([0, 2, 1]),
                    axis=mybir.AxisListType.X, op=mybir.AluOpType.add)
            nc.vector.reciprocal(out=Dg[:, 0:nb, :], in_=Dg[:, 0:nb, :])
            nc.vector.tensor_tensor(
                out=O[:, 0:nb, :].rearrange("p b (h f) -> p b h f", h=H),
                in0=O[:, 0:nb, :].rearrange("p b (h f) -> p b h f", h=H),
                in1=Dg[:, 0:nb, :].unsqueeze(3).to_broadcast(
                    [128, nb, H, F]),
                op=mybir.AluOpType.mult)
            nc.vector.tensor_tensor(
                out=O[:, 0:nb, :], in0=O[:, 0:nb, :],
                in1=bskG[:, 0:nb, 0:HF], op=mybir.AluOpType.add)

            # ELU = (max(O,0)-1) + exp(min(O,0)), group-wide
            nc.vector.tensor_scalar(
                out=T1[:, 0:nb, :], in0=O[:, 0:nb, :],
                scalar1=0.0, scalar2=-1.0,
                op0=mybir.AluOpType.max, op1=mybir.AluOpType.add)
            nc.vector.tensor_scalar(
                out=O[:, 0:nb, :], in0=O[:, 0:nb, :],
                scalar1=0.0, scalar2=None, op0=mybir.AluOpType.min)
            nc.scalar.activation(out=O[:, 0:nb, :], in_=O[:, 0:nb, :],
                                 func=mybir.ActivationFunctionType.Exp)
            nc.vector.tensor_tensor(out=T1[:, 0:nb, :], in0=T1[:, 0:nb, :],
                                    in1=O[:, 0:nb, :],
                                    op=mybir.AluOpType.add)
            nc.sync.dma_start(
                out=out_d[b0 * 128:(b0 + nb) * 128, :].rearrange(
                    "(b p) e -> p b e", p=128),
                in_=T1[:, 0:nb, :])

    nc.compile()
    return nc


def kernel(x, edge_index, W_proj, W_skip, a_src, a_tgt):
    common, per_core = _host_prep(x, edge_index, W_proj, W_skip, a_src, a_tgt)
    key = "prog"
    if key not in _COMPILED:
        _COMPILED[key] = _build_program(common["W"])
    nc = _COMPILED[key]

    in_maps = []
    for c in range(N_CORES):
        pc = per_core[c]
        in_maps.append({
            "xT": common["xT"],
            "xTperm": pc["xTperm"],
            "pack0": common["pack0"],
            "pack2": common["pack2"],
            "idxs": pc["idxs"],
            "gcnt": pc["gcnt"],
        })
    trace = bool(int(os.environ.get("GAT_TRACE", "0")))
    res = run_bass_kernel_spmd(nc, in_maps, list(range(N_CORES)),
                               trace=trace)
    if trace:
        kernel.last_exec_time_ns = res.exec_time_ns
        kernel.last_mean_exec_time_ns = res.mean_exec_time_ns

    out = np.empty((N_NODES, HF), np.float32)
    for c in range(N_CORES):
        o = res.results[c]["out"]  # [12544, 128] in rank order
        perm = per_core[c]["perm"]
        out[c * TGT_PER_CORE + perm] = o[:TGT_PER_CORE]
    return out


kernel.last_exec_time_ns = None
kernel.last_mean_exec_time_ns = None
